# revision 1
# baseline (speedup 1.0000x reference)
"""GroupedLinear Trainium2 kernel.

Math: out[b, g*R + r] = sum_s x[b, perm[g, s]] * W[g, r, s] + bias[g, r]
with B=8192, C=4096, G=16, S=256, R=512.

Strategy
--------
* Host: apply the channel permutation while building a transposed,
  group-contiguous activation tensor xT[(g,s), b] in bf16, and transpose the
  per-group weights to Wt[(g,k), s, r] in bf16. The per-core batch shard is a
  column slice of xT (data-parallel over batch, weights replicated).
* Device (per core, batch shard of 1024):
  For each group g, r-tile (128 rows of R) and batch-half (512 cols):
  psum[r, b] = sum over 2 K-chunks of Wt_chunk.T @ xT_chunk  (bf16 matmul,
  fp32 accumulate, N=512 = one PSUM bank; k-outer/bh-inner order so each
  LDWEIGHTS feeds two matmuls). The PSUM->SBUF bf16 downcast copies split
  50/50 between the Vector (DVE) and Scalar (ACT) engines — the copy stream
  is otherwise the bottleneck engine. Input loads ride the 4 SWDGE queues
  (gpsimd), output stores the HWDGE queues (sync), so loads and stores
  don't contend for descriptor slots.
* Host: outT shards are transposed/upcast into the full fp32 output with the
  bias folded in during assembly (a bias AP on-device costs an extra engine
  op per tile; the host add is free relative to the transpose pass).

Cost-model prediction: ~62us/core (PE floor 57us, DMA floor 55us).
"""

import numpy as np
import ml_dtypes

import concourse.bass as bass
import concourse.mybir as mybir
import concourse.tile as tile
from concourse import bacc
from concourse.bass_utils import run_bass_kernel_spmd

B, C, G, S, R = 8192, 4096, 16, 256, 512
N_CORES = 8
BC = B // N_CORES          # 1024 batch columns per core
KCH = S // 128             # 2 contraction chunks per group
NCH = G * KCH              # 32 row-chunks of xT
RT = R // 128              # 4 r-tiles per group
NB = 512                   # matmul free dim (one PSUM bank of fp32)
BH = BC // NB              # 2 batch halves per core

BF16 = mybir.dt.bfloat16
F32 = mybir.dt.float32
OUT_DT = BF16              # on-device staging dtype for the output

_BASS_CACHE: dict = {}


def _build_bass(skip_store=False, skip_load=False, skip_pe=False, skip_copy=False):
    nc = bacc.Bacc(None, num_swdge_queues=4)
    xT = nc.declare_dram_parameter("xT", [NCH, 128, BC], BF16, isOutput=False)
    Wt = nc.declare_dram_parameter("Wt", [NCH, 128, R], BF16, isOutput=False)
    outT = nc.declare_dram_parameter("outT", [G * R, BC], OUT_DT, isOutput=True)

    with tile.TileContext(nc) as tc:
        with (
            tc.tile_pool(name="xp", bufs=3) as xp,
            tc.tile_pool(name="wp", bufs=3) as wp,
            tc.tile_pool(name="op", bufs=6) as op,
            tc.tile_pool(name="pp", bufs=8, space="PSUM") as pp,
        ):
            for g in range(G):
                xg = xp.tile([128, KCH * BC], BF16, tag="x")
                wg = wp.tile([128, KCH * R], BF16, tag="w")
                # one batched DMA per group per tensor: [k, 128, m] -> [128, (k m)]
                if not skip_load:
                    for k in range(KCH):
                        nc.gpsimd.dma_start(out=xg[:, k * BC:(k + 1) * BC], in_=xT[g * KCH + k])
                        nc.gpsimd.dma_start(out=wg[:, k * R:(k + 1) * R], in_=Wt[g * KCH + k])
                else:
                    nc.vector.memset(xg[:, :4], 0)
                    nc.vector.memset(wg[:, :4], 0)
                for rt in range(RT):
                    ot = op.tile([128, BC], OUT_DT, tag="o")
                    pss = []
                    for _bh in range(BH):
                        ps_t = pp.tile([128, NB], F32, tag="ps")
                        pss.append(ps_t)
                    # k outer / bh inner: each LDWEIGHTS feeds BH matmuls
                    if skip_pe:
                        for ps in pss:
                            nc.vector.memset(ps[:, :4], 0)
                    else:
                        for k in range(KCH):
                            for bh in range(BH):
                                nc.tensor.matmul(
                                    out=pss[bh][:],
                                    lhsT=wg[:, k * R + rt * 128: k * R + rt * 128 + 128],
                                    rhs=xg[:, k * BC + bh * NB: k * BC + (bh + 1) * NB],
                                    start=(k == 0),
                                    stop=(k == KCH - 1),
                                )
                    for bh in range(BH):
                        if not skip_copy:
                            dst = ot[:, bh * NB:(bh + 1) * NB]
                            if (rt * BH + bh) % 2 == 0:
                                nc.vector.tensor_copy(out=dst, in_=pss[bh][:])
                            else:
                                nc.scalar.copy(out=dst, in_=pss[bh][:])
                        else:
                            nc.vector.memset(ot[:, bh * NB: bh * NB + 4], 0)
                            nc.vector.memset(pss[bh][:, :4], 0)
                    if not skip_store:
                        nc.sync.dma_start(
                            out=outT[g * R + rt * 128: g * R + (rt + 1) * 128, :],
                            in_=ot[:],
                        )
    if not nc.is_finalized():
        nc.finalize()
    return nc


def _get_bass():
    if "nc" not in _BASS_CACHE:
        _BASS_CACHE["nc"] = _build_bass()
    return _BASS_CACHE["nc"]


def _prepare_inputs(x, W, b, perm):
    bf16 = ml_dtypes.bfloat16
    perm_flat = np.asarray(perm).reshape(-1)

    # Gather the permuted channels (within-row gather: cache friendly), cast
    # to bf16, then transpose to channel-major [C, B].
    xg = np.ascontiguousarray(x)[:, perm_flat].astype(bf16)   # [B, C]
    xT = np.ascontiguousarray(xg.T)                           # [C, B] bf16

    Wt = np.ascontiguousarray(np.asarray(W).transpose(0, 2, 1)).astype(bf16)
    Wt = Wt.reshape(NCH, 128, R)

    in_maps = []
    for c in range(N_CORES):
        xT_c = np.ascontiguousarray(xT[:, c * BC:(c + 1) * BC]).reshape(NCH, 128, BC)
        in_maps.append({"xT": xT_c, "Wt": Wt})
    return in_maps


def kernel(x, W, b, perm, _trace=False, _trace_kwargs=None):
    nc = _get_bass()
    in_maps = _prepare_inputs(x, W, b, perm)
    res = run_bass_kernel_spmd(
        nc, in_maps, list(range(N_CORES)),
        trace=_trace, **(_trace_kwargs or {}),
    )
    b_flat = np.asarray(b, dtype=np.float32).reshape(-1)
    out = np.empty((B, G * R), dtype=np.float32)
    for c in range(N_CORES):
        blk = res.results[c]["outT"].T.astype(np.float32)
        blk += b_flat[None, :]
        out[c * BC:(c + 1) * BC, :] = blk
    if _trace:
        return out, res
    return out

